# revision 1
# baseline (speedup 1.0000x reference)
"""Multi-head self-attention (B=2, S=2048, E=1024, H=16) on 8 TRN2 NeuronCores.

Sharding: core c handles batch b=c//4 and head group g=c%4 (4 heads each).
 - QKV projections are head-sharded (each core computes Q/K/V only for its
   4 heads, over all 2048 tokens of its batch) -> no K/V exchange needed.
 - Attention (scores -> exp -> AV) is fully local per core.
 - One small AllToAll per head re-shards the attention output from
   head-sharding to token-sharding (within each batch's 4-core group;
   8-way mesh AllToAll with duplicated cross-batch blocks since 4-core
   AllToAll is unsupported).
 - Output projection (Wo) then runs token-sharded, producing complete
   output rows; the host just transposes/concatenates.

Everything feature-major ("transposed") on device: x, Q, K are [dim, tok]
so the PE's partition-contraction works without any on-device transposes;
weights are pre-transposed on the host. V is tok-major for the AV matmul,
with a fused ones-column producing softmax row-sums for free.

Matmuls run in float32r (TF32, 1 cycle/row at moving free>=256, vs 4 for
fp32). Inputs are pre-rounded to TF32 on the host so DMA-loaded operands
are valid fp32r. Softmax skips the max-subtraction (logits are ~N(0,1),
bounded ~|6|, exp is safe in fp32) and folds the 1/sqrt(64) scale into the
ACT exp instruction; normalization is deferred past the AV matmul using
the fused row-sums.
"""

import numpy as np

import concourse.bass as bass
import concourse.mybir as mybir
from concourse import tile, bacc
from concourse.tile import add_dep_helper
from concourse.bass_utils import run_bass_kernel_spmd

B = 2
S = 2048
E = 1024
H = 16
DH = 64

NCORES = 8
GH = 4          # heads per core
GD = GH * DH    # 256 feature dims per core
TOK = S         # tokens per core (its whole batch element)
QB = 512        # q-block (moving free dim)
NQB = TOK // QB         # 4
NKT = TOK // 128        # 16 k-tiles
NET = E // 128          # 8 e-tiles
SCALE = 1.0 / np.sqrt(DH)

F32 = mybir.dt.float32
F32R = mybir.dt.float32r
FP = mybir.ActivationFunctionType


def _tf32_round(a: np.ndarray) -> np.ndarray:
    b = np.ascontiguousarray(a, dtype=np.float32).view(np.uint32)
    r = (b + np.uint32(0x1000) + ((b >> np.uint32(13)) & np.uint32(1))) & np.uint32(0xFFFFE000)
    return r.view(np.float32)


def build_nc(reps: int = 1):
    nc = bacc.Bacc("TRN2", target_bir_lowering=False, debug=False, num_devices=NCORES)

    xt = nc.dram_tensor("xt", [E, TOK], F32R, kind="ExternalInput")       # x[b].T
    wqt = nc.dram_tensor("wqt", [E, GD], F32R, kind="ExternalInput")      # Wq.T cols for group
    wkt = nc.dram_tensor("wkt", [E, GD], F32R, kind="ExternalInput")
    wvt = nc.dram_tensor("wvt", [E, GD], F32R, kind="ExternalInput")
    wot = nc.dram_tensor("wot", [E, E], F32R, kind="ExternalInput")       # Wo.T full
    bq = nc.dram_tensor("bq", [GD], F32, kind="ExternalInput")
    bk = nc.dram_tensor("bk", [GD], F32, kind="ExternalInput")
    bv = nc.dram_tensor("bv", [GD], F32, kind="ExternalInput")
    bo = nc.dram_tensor("bo", [E], F32, kind="ExternalInput")
    # per-core batch masks: mlo = 1.0 on batch-0 cores, mhi = 1.0 on batch-1
    mlo = nc.dram_tensor("mlo", [65], F32, kind="ExternalInput")
    mhi = nc.dram_tensor("mhi", [65], F32, kind="ExternalInput")
    sel = nc.dram_tensor("sel", [4, 256], F32R, kind="ExternalInput")
    yt = nc.dram_tensor("yt", [E, QB], F32, kind="ExternalOutput")        # out rows, transposed

    with tile.TileContext(nc) as tc:
        with (
            tc.tile_pool(name="weights", bufs=1) as wp,
            tc.tile_pool(name="persist", bufs=1) as pp,
            tc.tile_pool(name="xt", bufs=10) as xp,
            tc.tile_pool(name="at", bufs=3) as ap_,
            tc.tile_pool(name="ot", bufs=3) as op_,
            tc.tile_pool(name="otf", bufs=1) as fp_,
            tc.tile_pool(name="yt", bufs=2) as yp,
            tc.tile_pool(name="dram", bufs=1, space="DRAM") as dp,
        ):
            # ---- persistent weights/biases in SBUF ----
            wq_t = [wp.tile([128, GD], F32R, name=f"wq{e}", tag=f"wq{e}") for e in range(NET)]
            wk_t = [wp.tile([128, GD], F32R, name=f"wk{e}", tag=f"wk{e}") for e in range(NET)]
            wv_t = [wp.tile([128, GD], F32R, name=f"wv{e}", tag=f"wv{e}") for e in range(NET)]
            wo_t = [wp.tile([128, E], F32R, name=f"wo{k}", tag=f"wo{k}") for k in range(NET)]
            for e in range(NET):
                nc.sync.dma_start(wq_t[e][:], wqt[e * 128:(e + 1) * 128, :])
                nc.sync.dma_start(wk_t[e][:], wkt[e * 128:(e + 1) * 128, :])
                nc.sync.dma_start(wv_t[e][:], wvt[e * 128:(e + 1) * 128, :])
                nc.sync.dma_start(wo_t[e][:], wot[e * 128:(e + 1) * 128, :])

            bq_t = [pp.tile([128, 1], F32, name=f"bq{d}", tag=f"bq{d}") for d in range(2)]
            bk_t = [pp.tile([128, 1], F32, name=f"bk{d}", tag=f"bk{d}") for d in range(2)]
            bo_t = [pp.tile([128, 1], F32, name=f"bo{e}", tag=f"bo{e}") for e in range(NET)]
            for d in range(2):
                nc.sync.dma_start(
                    bq_t[d][:], bq[d * 128:(d + 1) * 128].rearrange("(p one) -> p one", one=1))
                nc.sync.dma_start(
                    bk_t[d][:], bk[d * 128:(d + 1) * 128].rearrange("(p one) -> p one", one=1))
            for e in range(NET):
                nc.sync.dma_start(
                    bo_t[e][:], bo[e * 128:(e + 1) * 128].rearrange("(p one) -> p one", one=1))
            # bv broadcast across partitions: [GD] -> [128, GD]
            bv_t = pp.tile([128, GD], F32, name="bv", tag="bv")
            nc.gpsimd.dma_start(bv_t[:], bv.ap().partition_broadcast(128))
            mlo_t = pp.tile([65, 1], F32, name="mlo", tag="mlo")
            mhi_t = pp.tile([65, 1], F32, name="mhi", tag="mhi")
            nc.sync.dma_start(mlo_t[:], mlo.rearrange("(p one) -> p one", one=1))
            nc.sync.dma_start(mhi_t[:], mhi.rearrange("(p one) -> p one", one=1))
            ones_r = pp.tile([1, 64], F32R, name="ones_r", tag="ones_r")

            ones_f32 = pp.tile([128, 64], F32, name="ones_f32", tag="ones_f32")
            nc.vector.memset(ones_f32[:], 1.0)
            nc.vector.tensor_copy(ones_r[:], ones_f32[0:1, :])
            # selector matrices: sel_r[:, g*64:(g+1)*64] has ones in row g, else 0
            sel_r = pp.tile([4, 4 * 64], F32R, name="sel_r", tag="sel_r")
            nc.sync.dma_start(sel_r[:], sel[:])

            # persistent activations
            qt_sb = [pp.tile([128, TOK], F32R, name=f"qt{d}", tag=f"qt{d}") for d in range(2)]
            kt_sb = [pp.tile([128, TOK], F32R, name=f"kt{d}", tag=f"kt{d}") for d in range(2)]
            # V tok-major, packed [v_h | 1] per head: 65 cols per head
            vp_sb = [pp.tile([128, GH * 65], F32R, name=f"vp{t}", tag=f"vp{t}") for t in range(NKT)]
            for t in range(NKT):
                for h in range(GH):
                    nc.vector.tensor_copy(
                        vp_sb[t][:, h * 65 + 64:h * 65 + 65], ones_f32[:, 0:1])

            # A2A bounce buffers (per head): 65 rows = [O_unnorm ; rowsum]
            a2a_in = [dp.tile([NCORES, 65, QB], F32R, name=f"a2ain{h}", tag=f"a2ain{h}") for h in range(GH)]
            a2a_out = [dp.tile([NCORES, 65, QB], F32R, name=f"a2aout{h}", tag=f"a2aout{h}") for h in range(GH)]

            for _ in range(reps):
                # ================= Phase 1: QKV projections =================
                with tc.tile_pool(name="ps_qkv", bufs=1, space="PSUM") as ps_qkv:
                    for tb in range(NQB):
                        xts = []
                        for e in range(NET):
                            xt_t = xp.tile([128, QB], F32R, name="xt", tag="xt")
                            nc.sync.dma_start(xt_t[:], xt[e * 128:(e + 1) * 128, tb * QB:(tb + 1) * QB])
                            xts.append(xt_t)
                        # K/V before Q so attention's inputs complete earliest
                        for nm, (w_t, b_t, dst) in (("k", (wk_t, bk_t, kt_sb)), ("q", (wq_t, bq_t, qt_sb))):
                            for d in range(2):
                                ps = ps_qkv.tile([128, QB], F32, name=f"{nm}{d}", tag=f"{nm}{d}")
                                for e in range(NET):
                                    nc.tensor.matmul(
                                        ps[:], w_t[e][:, d * 128:(d + 1) * 128], xts[e][:],
                                        start=(e == 0), stop=(e == NET - 1))
                                nc.vector.tensor_scalar_add(
                                    dst[d][:, tb * QB:(tb + 1) * QB], ps[:], b_t[d][:])
                            if nm == "k":
                                # V: tok-major [tok, dh]
                                for vt in range(4):
                                    ps = ps_qkv.tile([128, GD], F32, name=f"v{vt % 2}", tag=f"v{vt % 2}")
                                    for e in range(NET):
                                        nc.tensor.matmul(
                                            ps[:], xts[e][:, vt * 128:(vt + 1) * 128], wv_t[e][:],
                                            start=(e == 0), stop=(e == NET - 1))
                                    t = tb * 4 + vt
                                    dst2 = vp_sb[t][:].rearrange("p (h c) -> p h c", h=GH)[:, :, 0:64]
                                    nc.vector.tensor_tensor(
                                        dst2, ps[:].rearrange("p (h c) -> p h c", h=GH),
                                        bv_t[:].rearrange("p (h c) -> p h c", h=GH),
                                        op=mybir.AluOpType.add)

                # ================= Phase 2: attention =================
                with (
                    tc.tile_pool(name="ps_s", bufs=2, space="PSUM") as ps_s,
                    tc.tile_pool(name="ps_av", bufs=2, space="PSUM") as ps_av,
                ):
                    cc_inst = {}
                    # k-tile groups sized to the scores psum tile (3 banks)
                    GRPS = [(0, 3), (3, 3), (6, 3), (9, 3), (12, 3), (15, 1)]
                    for h in range(GH):
                        a2a_writers = []
                        d, p0 = h // 2, (h % 2) * 64
                        for qb in range(NQB):
                            av_ps = ps_av.tile([65, QB], F32, name="av", tag="av")
                            for g0, gn in GRPS:
                                s_ps = ps_s.tile([128, 3 * QB], F32, name="s", tag="s")
                                for ki in range(gn):
                                    kt = g0 + ki
                                    nc.tensor.matmul(
                                        s_ps[:, ki * QB:(ki + 1) * QB],
                                        kt_sb[d][p0:p0 + 64, kt * 128:(kt + 1) * 128],
                                        qt_sb[d][p0:p0 + 64, qb * QB:(qb + 1) * QB],
                                        start=True, stop=True)
                                at_t = ap_.tile([128, 3 * QB], F32R, name="at", tag="at")
                                nc.scalar.activation(
                                    at_t[:, 0:gn * QB], s_ps[:, 0:gn * QB],
                                    FP.Exp, scale=float(SCALE))
                                for ki in range(gn):
                                    kt = g0 + ki
                                    nc.tensor.matmul(
                                        av_ps[:],
                                        vp_sb[kt][:, h * 65:h * 65 + 65],
                                        at_t[:, ki * QB:(ki + 1) * QB],
                                        start=(kt == 0), stop=(kt == NKT - 1))
                            # ship unnormalized [O_un ; rowsum]; receiver divides.
                            # Mask so the block is zero for wrong-batch receivers:
                            # block d is only valid when batch(sender)==batch(d).
                            ot_lo = op_.tile([65, QB], F32R, name="ot_lo", tag="ot_lo")
                            ot_hi = op_.tile([65, QB], F32R, name="ot_hi", tag="ot_hi")
                            with nc.allow_low_precision(reason="tf32 for fp32r matmul"):
                                nc.vector.tensor_scalar_mul(ot_lo[:], av_ps[:], mlo_t[:])
                                nc.vector.tensor_scalar_mul(ot_hi[:], av_ps[:], mhi_t[:])
                            a2a_writers.append(nc.sync.dma_start(a2a_in[h][qb], ot_lo[:]))
                            a2a_writers.append(nc.sync.dma_start(a2a_in[h][qb + 4], ot_hi[:]))
                        cc = nc.gpsimd.collective_compute(
                            "AllToAll", mybir.AluOpType.bypass,
                            replica_groups=[list(range(NCORES))],
                            ins=[a2a_in[h].opt()], outs=[a2a_out[h].opt()])
                        for w in a2a_writers:
                            add_dep_helper(cc.ins, w.ins, reason="collective waits on a2a input writes")
                        cc_inst[h] = cc

                # ================= Phase 3: output projection =================
                with (
                    tc.tile_pool(name="ps_y", bufs=2, space="PSUM") as ps_y,
                    tc.tile_pool(name="ps_rr", bufs=2, space="PSUM") as ps_rr,
                ):
                    # All-static receive: both candidate blocks are loaded; the
                    # wrong-batch one is zero (sender-masked), so add merges them.
                    # Rowsum rows (row 64 of each block) -> reciprocal -> PE
                    # ones-matmul broadcasts [1,512] across 64 partitions.
                    rcp4 = [fp_.tile([4, QB], F32R, name=f"rcp{head}", tag=f"rcp{head}")
                            for head in range(GH)]
                    for head in range(GH):
                        rs_lo = fp_.tile([4, QB], F32R, name="rs_lo", tag="rs_lo", bufs=2)
                        rs_hi = fp_.tile([4, QB], F32R, name="rs_hi", tag="rs_hi", bufs=2)
                        rd1 = nc.sync.dma_start(
                            rs_lo[:], a2a_out[head][0:4, 64:65, :].rearrange("b one f -> (b one) f"))
                        rd2 = nc.sync.dma_start(
                            rs_hi[:], a2a_out[head][4:8, 64:65, :].rearrange("b one f -> (b one) f"))
                        add_dep_helper(rd1.ins, cc_inst[head].ins, reason="rs read waits on collective")
                        add_dep_helper(rd2.ins, cc_inst[head].ins, reason="rs read waits on collective")
                        rsel = fp_.tile([4, QB], F32R, name="rsel", tag="rsel", bufs=2)
                        with nc.allow_low_precision(reason="tf32 for fp32r matmul"):
                            nc.vector.tensor_tensor(rsel[:], rs_lo[:], rs_hi[:],
                                                    op=mybir.AluOpType.add)
                            nc.vector.reciprocal(rcp4[head][:], rsel[:])

                    otf_t = [fp_.tile([128, QB], F32R, name=f"otf{k}", tag=f"otf{k}") for k in range(NET)]
                    for k in range(NET):
                        g_src, hh = k // 2, (k % 2) * 2
                        for half, head in ((0, hh), (1, hh + 1)):
                            ou_lo = fp_.tile([64, QB], F32R, name="ou_lo", tag="ou_lo", bufs=2)
                            ou_hi = fp_.tile([64, QB], F32R, name="ou_hi", tag="ou_hi", bufs=2)
                            rd1 = nc.sync.dma_start(ou_lo[:], a2a_out[head][g_src, 0:64, :])
                            rd2 = nc.sync.dma_start(ou_hi[:], a2a_out[head][4 + g_src, 0:64, :])
                            add_dep_helper(rd1.ins, cc_inst[head].ins, reason="otf read waits on collective")
                            add_dep_helper(rd2.ins, cc_inst[head].ins, reason="otf read waits on collective")
                            rr_ps = ps_rr.tile([64, QB], F32, name="rr", tag="rr")
                            nc.tensor.matmul(
                                rr_ps[:], sel_r[:, g_src * 64:(g_src + 1) * 64],
                                rcp4[head][:], start=True, stop=True)
                            dst = otf_t[k][half * 64:(half + 1) * 64, :]
                            with nc.allow_low_precision(reason="tf32 for fp32r matmul"):
                                nc.vector.tensor_tensor(dst, ou_lo[:], ou_hi[:],
                                                        op=mybir.AluOpType.add)
                                nc.vector.tensor_tensor(dst, dst, rr_ps[:],
                                                        op=mybir.AluOpType.mult)
                    # k-order matches A2A arrival: heads 0/1 (even k) land first
                    korder = [0, 2, 4, 6, 1, 3, 5, 7]
                    for e in range(NET):
                        ps = ps_y.tile([128, QB], F32, name="y", tag="y")
                        for i, k in enumerate(korder):
                            nc.tensor.matmul(
                                ps[:], wo_t[k][:, e * 128:(e + 1) * 128], otf_t[k][:],
                                start=(i == 0), stop=(i == NET - 1))
                        y_t = yp.tile([128, QB], F32, name="yt", tag="yt")
                        nc.vector.tensor_scalar_add(y_t[:], ps[:], bo_t[e][:])
                        nc.sync.dma_start(yt[e * 128:(e + 1) * 128, :], y_t[:])

    nc.compile()
    return nc


_CACHE = {}


def _get_nc(reps: int = 1):
    if reps not in _CACHE:
        _CACHE[reps] = build_nc(reps)
    return _CACHE[reps]


def make_in_maps(x, Wq, bq, Wk, bk, Wv, bv, Wo, bo):
    x = np.asarray(x, np.float32)
    xts = [_tf32_round(np.ascontiguousarray(x[b].T)) for b in range(B)]
    wqt = _tf32_round(np.ascontiguousarray(np.asarray(Wq, np.float32).T))
    wkt = _tf32_round(np.ascontiguousarray(np.asarray(Wk, np.float32).T))
    wvt = _tf32_round(np.ascontiguousarray(np.asarray(Wv, np.float32).T))
    wot = _tf32_round(np.ascontiguousarray(np.asarray(Wo, np.float32).T))
    bq = np.asarray(bq, np.float32); bk = np.asarray(bk, np.float32)
    bv = np.asarray(bv, np.float32); bo = np.asarray(bo, np.float32)
    in_maps = []
    for c in range(NCORES):
        b, g = c // 4, c % 4
        sl = slice(g * GD, (g + 1) * GD)
        in_maps.append({
            "mlo": np.full(65, 1.0 if b == 0 else 0.0, np.float32),
            "mhi": np.full(65, 1.0 if b == 1 else 0.0, np.float32),
            "sel": _SEL,
            "xt": xts[b],
            "wqt": np.ascontiguousarray(wqt[:, sl]),
            "wkt": np.ascontiguousarray(wkt[:, sl]),
            "wvt": np.ascontiguousarray(wvt[:, sl]),
            "wot": wot,
            "bq": np.ascontiguousarray(bq[sl]),
            "bk": np.ascontiguousarray(bk[sl]),
            "bv": np.ascontiguousarray(bv[sl]),
            "bo": bo,
        })
    return in_maps


_SEL = np.zeros((4, 256), np.float32)
for _g in range(4):
    _SEL[_g, _g * 64:(_g + 1) * 64] = 1.0


def kernel(x, Wq, bq, Wk, bk, Wv, bv, Wo, bo):
    nc = _get_nc(1)
    in_maps = make_in_maps(x, Wq, bq, Wk, bk, Wv, bv, Wo, bo)
    res = run_bass_kernel_spmd(nc, in_maps, list(range(NCORES)))
    out = np.empty((B, S, E), np.float32)
    for c in range(NCORES):
        b, g = c // 4, c % 4
        out[b, g * QB:(g + 1) * QB, :] = res.results[c]["yt"].T
    return out

